# revision 28
# baseline (speedup 1.0000x reference)
"""Trainium2 Bass kernel for MiniTriangularUpdate.

Reference computation (per batch b):
  h  = layernorm(x)                                 # (N, N, D), ln affine = identity
  h  = (h @ w_pin.T) * sigmoid(h @ w_gin.T)         # gated down-proj, still D
  h *= mask[..., None]                              # mask is all-ones -> skipped
  a1, b1, a2, b2 = split(h, 4, axis=-1)             # (N, N, D/4) each
  x1[i,j,d] = sum_k a1[i,k,d] * b1[j,k,d]           # outgoing triangle
  x2[i,j,d] = sum_k a2[k,i,d] * b2[k,j,d]           # incoming triangle
  t  = concat([x1, x2], -1)                         # (N, N, D/2)
  t  = layernorm(t)                                 # ln affine = identity
  out = (t @ w_pout.T) * sigmoid(t @ w_gout.T)      # gated up-proj back to D

Sharding: 8 cores = 4 batches x 2 row-halves. Each core receives the full
(row+col permuted) batch pair-rep so that its output rows are always rows
0..127 of its local problem; the permutation (swap of row/col halves for the
second core of each batch) commutes with everything (LN / projections are
per-token, both einsums contract over a full axis).

Device token order is column-half-major: token (r, q) -> n = (q//128)*32768
+ r*128 + (q%128), so h_q0/h_q1 hold the q<128 / q>=128 column halves with
h_qh[p, r, c] = h of token (r, qh*128+p).

LN mean-subtraction is folded into the projection weights on the host
(W' = W - rowsum(W)/fan_in), for BOTH the input LN and the output LN, so on
device LN reduces to a per-token 1/sqrt(var+eps) scale. Variance comes from
one bn_stats per 128-token group; the even/odd partial stats are combined
with a handful of batched elementwise ops (replacing per-group bn_aggr), and
sqrt runs on ACT batched 64 groups at a time to amortize its table load.

All transposes are batched xbar DMA transposes ([128, S*128] -> [128, S, 128]
with out[p, s, c] = in[c, s*128+p], S up to 16), issued ONLY from nc.sync:
issuing xbar transposes concurrently from both HWDGE rings (SP + ACT) was
observed to corrupt transpose payloads non-deterministically. Plain loads /
stores ride GpSimd SWDGE (with f32->bf16 cast folded into the x load), and
the x2 operand staging copies are spread across GpSimd/DVE/ACT.

Within each rs batch, instructions are emitted stage-major (all scales, then
all transposes, then matmul->sigmoid->gate chains) so no engine queue stalls
head-of-line on a dependency of a later stage.
"""

import numpy as np

import concourse.bass as bass
import concourse.mybir as mybir
import concourse.tile as tile
from concourse.bass_utils import run_bass_kernel_spmd
from concourse.vector_clock import ScopedClock

# ---------------------------------------------------------------------------
# The walrus build in this container rejects instructions carrying more than
# 2 sync-wait commands ("Too many sync wait commands"), but Tile's semaphore
# pass freely attaches 3-10 waits per instruction. Post-process the BIR JSON
# just before compilation: hoist excess semaphore waits onto NoOp
# instructions inserted immediately before the over-limit instruction on the
# same engine (same-engine program order makes this semantically identical).
# ---------------------------------------------------------------------------
import orjson as _orjson

_MAX_INST_WAITS = 1


def _split_excess_waits(bir_json, max_waits=_MAX_INST_WAITS):
    if isinstance(bir_json, str):
        bir_json = bir_json.encode()
    m = _orjson.loads(bir_json)
    ctr = 0
    for fn in m.get("functions", []):
        for blk in fn.get("blocks", []):
            insts = blk.get("instructions", [])
            out = []
            changed = False
            for inst in insts:
                si = inst.get("sync_info")
                waits = (si or {}).get("on_wait") or []
                sem_w = [w for w in waits if w.get("sync_type") == "semaphore"]
                other_w = [w for w in waits if w.get("sync_type") != "semaphore"]
                budget = max_waits - len(other_w)
                if len(sem_w) > budget:
                    keep = sem_w[: max(budget, 0)]
                    extra = sem_w[max(budget, 0):]
                    for i in range(0, len(extra), max_waits):
                        ctr += 1
                        out.append(
                            {
                                "debug": inst.get("debug", 0),
                                "engine": inst["engine"],
                                "ins": [],
                                "outs": [],
                                "name": f"I-wsplit-{ctr}",
                                "opcode": "NoOp",
                                "sync_info": {
                                    "on_wait": extra[i : i + max_waits],
                                    "on_update": [],
                                },
                            }
                        )
                    si["on_wait"] = other_w + keep
                    changed = True
                out.append(inst)
            if changed:
                blk["instructions"] = out
    return _orjson.dumps(m)


def _install_compile_patch():
    import concourse.bass_utils as _bu
    import concourse.bass2jax as _b2j

    if getattr(_bu, "_wsplit_patched", False):
        return
    orig = _bu.compile_bir_kernel

    def patched(bir_json, tmpdir, neff_name="file.neff"):
        return orig(_split_excess_waits(bir_json), tmpdir, neff_name)

    _bu.compile_bir_kernel = patched
    _b2j.compile_bir_kernel = patched
    _bu._wsplit_patched = True


_install_compile_patch()

F32 = mybir.dt.float32
BF16 = mybir.dt.bfloat16
AF = mybir.ActivationFunctionType
ALU = mybir.AluOpType

B, N, D = 4, 256, 128
H = D // 2          # 64 triangle channels
Q = D // 4          # 32 channels per einsum operand
NT = N * N          # tokens per batch (65536)
EPS = 1e-5
N_CORES = 8

TOK = 2048          # tokens per P1 tile
NG1 = NT // TOK     # 32 P1 tiles
SB = 16             # 128-token stat groups per P1 tile
RSB = 4             # P1 tiles per rs batch (64 groups -> one sqrt)

# 1-wait-per-instruction splitting for the TileContext exit drain: the
# walrus build in this container rejects instructions carrying >2 sem waits.
_MAXW = 1


class _TC(tile.TileContext):
    def _drain_and_barrier(self, tick_clock, wait_clock):
        nc = self.nc
        probe = nc.sync.nop(nofuse=True)
        wait_clock.add_sem_waits(
            probe.ins, ScopedClock({None: tick_clock.global_clock})
        )
        si = probe.ins.sync_info
        waits = list(si.on_wait) if si is not None else []
        if len(waits) > _MAXW:
            probe.ins.sync_info = mybir.SyncInfo(
                on_wait=waits[:_MAXW], on_update=list(si.on_update)
            )
            rest = waits[_MAXW:]
            for i in range(0, len(rest), _MAXW):
                w = nc.sync.nop(nofuse=True)
                w.ins.sync_info = mybir.SyncInfo(
                    on_wait=rest[i : i + _MAXW], on_update=[]
                )
        nc.sync.drain()
        nc.all_engine_barrier()
        popped = nc._tile_sem_poison_stack.pop()
        assert popped is self._sem_poison
        nc.clear_and_free_semaphores(list(self.sems.allocated().values()))
        nc.all_engine_barrier()


def _rs_from_stats(nc, pool, st, rs, ngrp, inv_n, eps_sb):
    """rs[:, :ngrp] = 1/sqrt(var + eps) from bn_stats partials st[128, ngrp, 6].

    bn_stats emits (count, mean, count*var) for even and odd elements; the
    exact combine for equal halves is
      var = (cv_e + cv_o) * inv_n + ((m_e - m_o)/2)^2.
    """
    d = pool.tile([128, ngrp], F32, tag="rsd")
    nc.vector.tensor_sub(out=d, in0=st[:, :, 1], in1=st[:, :, 4])
    nc.vector.tensor_scalar_mul(out=d, in0=d, scalar1=0.5)
    sq = pool.tile([128, ngrp], F32, tag="rssq")
    nc.vector.tensor_mul(out=sq, in0=d, in1=d)
    cv = pool.tile([128, ngrp], F32, tag="rscv")
    nc.vector.tensor_add(out=cv, in0=st[:, :, 2], in1=st[:, :, 5])
    var = pool.tile([128, ngrp], F32, tag="rsvar")
    nc.vector.scalar_tensor_tensor(
        out=var, in0=cv, scalar=inv_n, in1=sq, op0=ALU.mult, op1=ALU.add
    )
    nc.scalar.activation(out=rs, in_=var, func=AF.Sqrt, bias=eps_sb, scale=1.0)
    nc.vector.reciprocal(out=rs, in_=rs)


def _build(ctx, tc):
    nc = tc.nc

    # x_pre[p, (g, s, c)] = x token (g*1024 + s*128 + p) in column-half-major
    # order (host-shuffled), channel c.
    x_rows = nc.dram_tensor("x_rows", (128, NT * D // 128), F32, kind="ExternalInput").ap()
    w_pin = nc.dram_tensor("w_pin_t", (D, D), BF16, kind="ExternalInput").ap()
    w_gin = nc.dram_tensor("w_gin_t", (D, D), BF16, kind="ExternalInput").ap()
    w_pout = nc.dram_tensor("w_pout_t", (H, D), BF16, kind="ExternalInput").ap()
    w_gout = nc.dram_tensor("w_gout_t", (H, D), BF16, kind="ExternalInput").ap()
    out_cm = nc.dram_tensor("out_cm", (D, NT // 2), F32, kind="ExternalOutput").ap()

    persist = ctx.enter_context(tc.tile_pool(name="persist", bufs=1))
    # h_qh[p, r, c] = gated-h of token (r, qh*128 + p), channel c.
    h_q0 = persist.tile([128, N, D], BF16)
    h_q1 = persist.tile([128, N, D], BF16)
    w_pin_sb = persist.tile([D, D], BF16)
    w_gin_sb = persist.tile([D, D], BF16)
    w_pout_sb = persist.tile([H, D], BF16)
    w_gout_sb = persist.tile([H, D], BF16)
    eps_sb = persist.tile([128, 1], F32)
    nc.sync.dma_start(out=w_pin_sb, in_=w_pin)
    nc.sync.dma_start(out=w_gin_sb, in_=w_gin)
    nc.sync.dma_start(out=w_pout_sb, in_=w_pout)
    nc.sync.dma_start(out=w_gout_sb, in_=w_gout)
    nc.vector.memset(eps_sb, EPS)

    # ---------------- Phase 1: LN + gated down-projection ----------------
    with (
        tc.tile_pool(name="p1b", bufs=RSB + 2) as p1b,
        tc.tile_pool(name="p1st", bufs=2) as p1st,
        tc.tile_pool(name="p1rs", bufs=2) as p1rs,
        tc.tile_pool(name="p1t", bufs=2 * RSB - 1) as p1t,
        tc.tile_pool(name="p1h", bufs=2) as p1h,
        tc.tile_pool(name="p1hg", bufs=3) as p1hg,
        tc.tile_pool(name="p1p", bufs=2, space="PSUM") as p1p,
    ):
        def emit_proj(items):
            # stage B: matmul -> sigmoid/ppb -> gate -> h write, one rs batch
            for gg, xT in items:
                xTf = xT.rearrange("p s c -> p (s c)")
                hg = p1hg.tile([128, SB, D], BF16, tag="hg")
                hgf = hg.rearrange("p s c -> p (s c)")
                for u2 in range(2):
                    base = u2 * 1024
                    pp = p1p.tile([128, 1024], F32, tag="pp")
                    pg = p1p.tile([128, 1024], F32, tag="pg")
                    for hh in range(2):
                        so = slice(base + hh * 512, base + (hh + 1) * 512)
                        nc.tensor.matmul(
                            pp[:, hh * 512 : (hh + 1) * 512],
                            w_pin_sb, xTf[:, so], start=True, stop=True,
                        )
                    for hh in range(2):
                        so = slice(base + hh * 512, base + (hh + 1) * 512)
                        nc.tensor.matmul(
                            pg[:, hh * 512 : (hh + 1) * 512],
                            w_gin_sb, xTf[:, so], start=True, stop=True,
                        )
                    sg = p1h.tile([128, 1024], BF16, tag="sg")
                    nc.scalar.activation(out=sg, in_=pg, func=AF.Sigmoid)
                    ppb = p1h.tile([128, 1024], BF16, tag="ppb")
                    nc.scalar.activation(out=ppb, in_=pp, func=AF.Copy)
                    nc.vector.tensor_mul(
                        out=hgf[:, base : base + 1024], in0=ppb, in1=sg
                    )
                # tokens gg*2048.. are rows r = gg*16.. of half gg//16
                h_dst = h_q0 if gg < NG1 // 2 else h_q1
                r0 = (gg % (NG1 // 2)) * SB
                nc.sync.dma_start_transpose(
                    out=h_dst[:, r0 : r0 + SB, :], in_=hgf
                )

        sts = None
        xbs = []
        pend = None  # stage B of the previous rs batch, emitted one batch late
        for g in range(NG1):
            gb = g % RSB
            if gb == 0:
                sts = p1st.tile([128, SB * RSB, 6], F32, tag="st")
                xbs = []
            # f32 -> bf16 cast rides the SWDGE load
            xb = p1b.tile([128, SB, D], BF16, tag="xb")
            nc.gpsimd.dma_start(
                out=xb,
                in_=x_rows[:, g * TOK : (g + 1) * TOK].rearrange(
                    "p (s c) -> p s c", s=SB
                ),
            )
            xbs.append(xb)
            for s in range(SB):
                nc.vector.bn_stats(out=sts[:, gb * SB + s, :], in_=xb[:, s, :])
            if gb == RSB - 1:
                # stage A: rs, scales, xT transposes.  Emitting this BEFORE
                # the previous batch's projection chain puts the Sqrt ahead
                # of that batch's sigmoid burst in the ACT queue, so the
                # scales (DVE) unblock while the sigmoids still run.
                rs = p1rs.tile([128, SB * RSB], F32, tag="rs")
                _rs_from_stats(nc, p1rs, sts, rs, SB * RSB, 1.0 / D, eps_sb)
                rsv = rs.rearrange("p (s one) -> p s one", one=1)
                for gi in range(RSB):  # scale in place (one broadcast TT)
                    rb = rsv[:, gi * SB : (gi + 1) * SB, :].broadcast_to(
                        (128, SB, D)
                    )
                    nc.vector.tensor_mul(out=xbs[gi], in0=xbs[gi], in1=rb)
                items = []
                for gi in range(RSB):
                    xT = p1t.tile([128, SB, D], BF16, tag="xT")
                    nc.sync.dma_start_transpose(
                        out=xT, in_=xbs[gi].rearrange("p s c -> p (s c)")
                    )
                    items.append((g - (RSB - 1) + gi, xT))
                if pend is not None:
                    emit_proj(pend)
                pend = items
        emit_proj(pend)

    # ---------------- Phase 2: triangle matmuls ----------------
    hq = (h_q0, h_q1)
    with tc.tile_pool(name="p2tri", bufs=1) as p2tri:
        # tri[p, c, jh, i] = triangle-out channel c of token (i, jh*128 + p)
        tri = p2tri.tile([128, H, 2, 128], BF16)

        with (
            tc.tile_pool(name="p2s", bufs=3) as p2s,
            tc.tile_pool(name="p2t", bufs=3) as p2t,
            tc.tile_pool(name="p2e", bufs=3) as p2e,
            tc.tile_pool(name="p2p", bufs=4, space="PSUM") as p2p,
        ):
            # channels are processed 4 at a time so every xbar transpose
            # moves a [128, 1024] block (the per-op cost is ~fixed).
            def evac4(c_out, pss):
                ev = p2e.tile([128, 4, 256], BF16, tag="ev")
                for co, ps in enumerate(pss):
                    if co % 2 == 0:
                        nc.scalar.activation(out=ev[:, co, :], in_=ps,
                                             func=AF.Copy)
                    else:
                        nc.vector.tensor_copy(out=ev[:, co, :], in_=ps)
                nc.sync.dma_start_transpose(
                    out=tri[:, c_out : c_out + 4, :, :].rearrange(
                        "p c jh i -> p (c jh) i"
                    ),
                    in_=ev.rearrange("p a b -> p (a b)"),
                )

            for c4 in range(Q // 4):  # x1: a1=ch c, b1=ch Q+c
                os = []
                for co in range(4):
                    c = 4 * c4 + co
                    o1 = p2p.tile([128, 256], F32, tag="o1")
                    for kh in range(2):
                        nc.tensor.matmul(
                            o1,
                            hq[kh][:, 0:128, c],
                            hq[kh][:, 0:256, Q + c],
                            start=(kh == 0),
                            stop=(kh == 1),
                        )
                    os.append(o1)
                evac4(4 * c4, os)
            for c4 in range(Q // 4):  # x2: a2=ch 2Q+c, b2=ch 3Q+c
                # a2[k, i] lives at h_q0[p=i, r=k, 2Q+c]; b2[k, j] at
                # h_q(jh)[p=j%128, r=k, 3Q+c].  Stage contiguous copies
                # (spread over idle engines), then batched xbar transposes
                # put k on partitions.
                a2s = p2s.tile([128, 4, 256], BF16, tag="a2s")
                a2t = p2t.tile([128, 4, 2, 128], BF16, tag="a2t")  # [p,co,kb,i]
                for co in range(4):
                    src = h_q0[:, :, 2 * Q + 4 * c4 + co]
                    if co % 2 == 0:
                        nc.gpsimd.tensor_copy(out=a2s[:, co, :], in_=src)
                    else:
                        nc.vector.tensor_copy(out=a2s[:, co, :], in_=src)
                nc.sync.dma_start_transpose(
                    out=a2t.rearrange("p a b i -> p (a b) i"),
                    in_=a2s.rearrange("p a b -> p (a b)"),
                )
                b2t = p2t.tile([128, 4, 2, 2, 128], BF16, tag="b2t")  # [p,co,kb,jh,jl]
                for jh in range(2):
                    b2s = p2s.tile([128, 4, 256], BF16, tag="b2s")
                    for co in range(4):
                        src = hq[jh][:, :, 3 * Q + 4 * c4 + co]
                        if co % 2 == 0:
                            nc.gpsimd.tensor_copy(out=b2s[:, co, :], in_=src)
                        elif jh == 0:
                            nc.vector.tensor_copy(out=b2s[:, co, :], in_=src)
                        else:
                            nc.scalar.activation(
                                out=b2s[:, co, :], in_=src, func=AF.Copy
                            )
                    nc.sync.dma_start_transpose(
                        out=b2t[:, :, :, jh, :].rearrange("p a b i -> p (a b) i"),
                        in_=b2s.rearrange("p a b -> p (a b)"),
                    )
                for co in range(4):
                    o2 = p2p.tile([128, 256], F32, tag="o2")
                    for kb in range(2):
                        nc.tensor.matmul(
                            o2,
                            a2t[:, co, kb, :],
                            b2t[:, co, kb, :, :].rearrange("p a b -> p (a b)"),
                            start=(kb == 0),
                            stop=(kb == 1),
                        )
                    os2 = [o2] if co == 0 else os2 + [o2]
                evac4(Q + 4 * c4, os2)

        # ---------------- Phase 3: LN + gated up-projection ----------------
        ROWS = 8            # output rows per iteration (2048 tokens)
        NG3 = 128 // ROWS   # 16 iterations
        RSB3 = 4            # iterations per rs batch (64 groups)
        with (
            tc.tile_pool(name="p3st", bufs=2) as p3st,
            tc.tile_pool(name="p3rs", bufs=2) as p3rs,
            tc.tile_pool(name="p3n", bufs=3) as p3n,
            tc.tile_pool(name="p3r", bufs=2) as p3r,
            tc.tile_pool(name="p3o", bufs=2) as p3o,
            tc.tile_pool(name="p3ob", bufs=1) as p3ob,
            tc.tile_pool(name="p3p", bufs=2, space="PSUM") as p3p,
        ):
            tri_v = tri.rearrange("p c jh i -> p i jh c")
            GPI = 2 * ROWS  # stat groups per iteration
            sts3 = None
            for i8 in range(NG3):
                gb = i8 % RSB3
                if gb == 0:
                    sts3 = p3st.tile([128, GPI * RSB3, 6], F32, tag="st")
                for u in range(ROWS):
                    i = ROWS * i8 + u
                    for jh in range(2):
                        nc.vector.bn_stats(
                            out=sts3[:, gb * GPI + u * 2 + jh, :],
                            in_=tri_v[:, i, jh, :],
                        )
                if gb == RSB3 - 1:
                    rs3 = p3rs.tile([128, GPI * RSB3], F32, tag="rs")
                    _rs_from_stats(
                        nc, p3rs, sts3, rs3, GPI * RSB3, 1.0 / H, eps_sb
                    )
                    rsv = rs3.rearrange("p (s one) -> p s one", one=1)
                    hns = []
                    for ii in range(RSB3):
                        g3 = i8 - (RSB3 - 1) + ii
                        # hn[p, (u, jh), c] = normalized tri, channels 64..127
                        # garbage (never read: the matmuls contract K=64).
                        hn = p3n.tile([128, GPI, 128], BF16, tag="hn")
                        out4 = hn.rearrange("p (a b) c -> p a b c", b=2)[
                            :, :, :, 0:H
                        ]
                        in4 = tri_v[:, ROWS * g3 : ROWS * (g3 + 1), :, :]
                        rb = (
                            rsv[:, ii * GPI : (ii + 1) * GPI, :]
                            .broadcast_to((128, GPI, H))
                            .rearrange("p (a b) c -> p a b c", b=2)
                        )
                        nc.vector.tensor_mul(out=out4, in0=in4, in1=rb)
                        hns.append(hn)
                    rhss = []
                    for ii in range(RSB3):
                        rhs = p3r.tile([128, GPI, 128], BF16, tag="rhs")
                        nc.sync.dma_start_transpose(
                            out=rhs, in_=hns[ii].rearrange("p a c -> p (a c)")
                        )
                        rhss.append(rhs)
                    for ii in range(RSB3):
                        g3 = i8 - (RSB3 - 1) + ii
                        rhsf = rhss[ii].rearrange("p a c -> p (a c)")[0:H, :]
                        for u2 in range(2):
                            base = u2 * 1024
                            pp3 = p3p.tile([D, 1024], F32, tag="pp")
                            pg3 = p3p.tile([D, 1024], F32, tag="pg")
                            for hh in range(2):
                                so = slice(base + hh * 512, base + (hh + 1) * 512)
                                nc.tensor.matmul(
                                    pp3[:, hh * 512 : (hh + 1) * 512],
                                    w_pout_sb, rhsf[:, so],
                                    start=True, stop=True,
                                )
                            for hh in range(2):
                                so = slice(base + hh * 512, base + (hh + 1) * 512)
                                nc.tensor.matmul(
                                    pg3[:, hh * 512 : (hh + 1) * 512],
                                    w_gout_sb, rhsf[:, so],
                                    start=True, stop=True,
                                )
                            sg3 = p3o.tile([D, 1024], BF16, tag="sg")
                            nc.scalar.activation(out=sg3, in_=pg3, func=AF.Sigmoid)
                            ob = p3ob.tile([D, 1024], F32, tag="ob")
                            nc.vector.tensor_mul(out=ob, in0=pp3, in1=sg3)
                            nc.gpsimd.dma_start(
                                out=out_cm[
                                    :, g3 * 2048 + base : g3 * 2048 + base + 1024
                                ],
                                in_=ob,
                            )


_NC_CACHE = None


def _get_nc():
    global _NC_CACHE
    if _NC_CACHE is None:
        from contextlib import ExitStack

        nc = bass.Bass()
        with _TC(nc) as tc:
            with ExitStack() as ctx:
                _build(ctx, tc)
        _NC_CACHE = nc
    return _NC_CACHE


def kernel(
    x, mask, ln_in_w, ln_in_b, w_pin, w_gin, ln_out_w, ln_out_b, w_pout, w_gout,
    _spmd_kwargs=None,
):
    x = np.asarray(x, dtype=np.float32)
    w_pin = np.asarray(w_pin, dtype=np.float32)
    w_gin = np.asarray(w_gin, dtype=np.float32)
    w_pout = np.asarray(w_pout, dtype=np.float32)
    w_gout = np.asarray(w_gout, dtype=np.float32)

    # Fold LN mean-subtraction into the projection weights:
    #   LN(x) @ W.T == (x * rs) @ W'.T  with  W' = W - rowsum(W)/fan_in
    # (valid because both ln affines are identity and rs commutes).
    wp = w_pin - w_pin.sum(axis=1, keepdims=True) / D
    wg = w_gin - w_gin.sum(axis=1, keepdims=True) / D
    wpo = w_pout - w_pout.sum(axis=1, keepdims=True) / H
    wgo = w_gout - w_gout.sum(axis=1, keepdims=True) / H
    import ml_dtypes

    bf = lambda a: np.ascontiguousarray(a, dtype=ml_dtypes.bfloat16)
    w_common = {
        "w_pin_t": bf(wp.T),
        "w_gin_t": bf(wg.T),
        "w_pout_t": bf(wpo.T),
        "w_gout_t": bf(wgo.T),
    }

    in_maps = []
    for b in range(B):
        xb = np.ascontiguousarray(x[b])  # (N, N, D)
        xb_sw = np.ascontiguousarray(
            xb[np.r_[N // 2 : N, 0 : N // 2]][:, np.r_[N // 2 : N, 0 : N // 2]]
        )
        for xp in (xb, xb_sw):
            # column-half-major token order: n = (q//128)*32768 + r*128 + q%128
            x_ord = xp.reshape(N, 2, 128, D).transpose(1, 0, 2, 3)
            # device tile layout: x_pre[p, (g, s, c)], token = g*1024+s*128+p
            x_pre = np.ascontiguousarray(
                x_ord.reshape(NG1, SB, 128, D).transpose(2, 0, 1, 3)
            ).reshape(128, NT * D // 128)
            in_maps.append({"x_rows": x_pre, **w_common})

    nc = _get_nc()
    res = run_bass_kernel_spmd(
        nc, in_maps, core_ids=list(range(N_CORES)), **(_spmd_kwargs or {})
    )

    out = np.empty((B, N, N, D), dtype=np.float32)
    roll = np.r_[N // 2 : N, 0 : N // 2]
    for b in range(B):
        o0 = res.results[2 * b]["out_cm"].reshape(D, N // 2, N)
        o1 = res.results[2 * b + 1]["out_cm"].reshape(D, N // 2, N)
        out[b, : N // 2] = o0.transpose(1, 2, 0)
        # roll is an involution, so reorder columns directly
        out[b, N // 2 :] = o1.transpose(1, 2, 0)[:, roll, :]
    kernel._last_results = res
    return out


# revision 29
# speedup vs baseline: 1.0764x; 1.0764x over previous
"""Trainium2 Bass kernel for MiniTriangularUpdate.

Reference computation (per batch b):
  h  = layernorm(x)                                 # (N, N, D), ln affine = identity
  h  = (h @ w_pin.T) * sigmoid(h @ w_gin.T)         # gated down-proj, still D
  h *= mask[..., None]                              # mask is all-ones -> skipped
  a1, b1, a2, b2 = split(h, 4, axis=-1)             # (N, N, D/4) each
  x1[i,j,d] = sum_k a1[i,k,d] * b1[j,k,d]           # outgoing triangle
  x2[i,j,d] = sum_k a2[k,i,d] * b2[k,j,d]           # incoming triangle
  t  = concat([x1, x2], -1)                         # (N, N, D/2)
  t  = layernorm(t)                                 # ln affine = identity
  out = (t @ w_pout.T) * sigmoid(t @ w_gout.T)      # gated up-proj back to D

Sharding: 8 cores = 4 batches x 2 row-halves. Each core receives the full
(row+col permuted) batch pair-rep so that its output rows are always rows
0..127 of its local problem; the permutation (swap of row/col halves for the
second core of each batch) commutes with everything (LN / projections are
per-token, both einsums contract over a full axis).

Device token order is column-half-major: token (r, q) -> n = (q//128)*32768
+ r*128 + (q%128), so h_q0/h_q1 hold the q<128 / q>=128 column halves with
h_qh[p, r, c] = h of token (r, qh*128+p).

LN mean-subtraction is folded into the projection weights on the host
(W' = W - rowsum(W)/fan_in), for BOTH the input LN and the output LN, so on
device LN reduces to a per-token 1/sqrt(var+eps) scale. Variance comes from
one bn_stats per 128-token group; the even/odd partial stats are combined
with a handful of batched elementwise ops (replacing per-group bn_aggr), and
sqrt runs on ACT batched 64 groups at a time to amortize its table load.

All transposes are batched xbar DMA transposes ([128, S*128] -> [128, S, 128]
with out[p, s, c] = in[c, s*128+p], S up to 16), issued ONLY from nc.sync:
issuing xbar transposes concurrently from both HWDGE rings (SP + ACT) was
observed to corrupt transpose payloads non-deterministically. Plain loads /
stores ride GpSimd SWDGE (with f32->bf16 cast folded into the x load), and
the x2 operand staging copies are spread across GpSimd/DVE/ACT.

Within each rs batch, instructions are emitted stage-major (all scales, then
all transposes, then matmul->sigmoid->gate chains) so no engine queue stalls
head-of-line on a dependency of a later stage.
"""

import numpy as np

import concourse.bass as bass
import concourse.mybir as mybir
import concourse.tile as tile
from concourse.bass_utils import run_bass_kernel_spmd
from concourse.vector_clock import ScopedClock

# ---------------------------------------------------------------------------
# The walrus build in this container rejects instructions carrying more than
# 2 sync-wait commands ("Too many sync wait commands"), but Tile's semaphore
# pass freely attaches 3-10 waits per instruction. Post-process the BIR JSON
# just before compilation: hoist excess semaphore waits onto NoOp
# instructions inserted immediately before the over-limit instruction on the
# same engine (same-engine program order makes this semantically identical).
# ---------------------------------------------------------------------------
import orjson as _orjson

_MAX_INST_WAITS = 1


def _split_excess_waits(bir_json, max_waits=_MAX_INST_WAITS):
    if isinstance(bir_json, str):
        bir_json = bir_json.encode()
    m = _orjson.loads(bir_json)
    ctr = 0
    for fn in m.get("functions", []):
        for blk in fn.get("blocks", []):
            insts = blk.get("instructions", [])
            out = []
            changed = False
            for inst in insts:
                si = inst.get("sync_info")
                waits = (si or {}).get("on_wait") or []
                sem_w = [w for w in waits if w.get("sync_type") == "semaphore"]
                other_w = [w for w in waits if w.get("sync_type") != "semaphore"]
                budget = max_waits - len(other_w)
                if len(sem_w) > budget:
                    keep = sem_w[: max(budget, 0)]
                    extra = sem_w[max(budget, 0):]
                    for i in range(0, len(extra), max_waits):
                        ctr += 1
                        out.append(
                            {
                                "debug": inst.get("debug", 0),
                                "engine": inst["engine"],
                                "ins": [],
                                "outs": [],
                                "name": f"I-wsplit-{ctr}",
                                "opcode": "NoOp",
                                "sync_info": {
                                    "on_wait": extra[i : i + max_waits],
                                    "on_update": [],
                                },
                            }
                        )
                    si["on_wait"] = other_w + keep
                    changed = True
                out.append(inst)
            if changed:
                blk["instructions"] = out
    return _orjson.dumps(m)


def _install_compile_patch():
    import concourse.bass_utils as _bu
    import concourse.bass2jax as _b2j

    if getattr(_bu, "_wsplit_patched", False):
        return
    orig = _bu.compile_bir_kernel

    def patched(bir_json, tmpdir, neff_name="file.neff"):
        return orig(_split_excess_waits(bir_json), tmpdir, neff_name)

    _bu.compile_bir_kernel = patched
    _b2j.compile_bir_kernel = patched
    _bu._wsplit_patched = True


_install_compile_patch()

F32 = mybir.dt.float32
BF16 = mybir.dt.bfloat16
AF = mybir.ActivationFunctionType
ALU = mybir.AluOpType

B, N, D = 4, 256, 128
H = D // 2          # 64 triangle channels
Q = D // 4          # 32 channels per einsum operand
NT = N * N          # tokens per batch (65536)
EPS = 1e-5
N_CORES = 8

TOK = 2048          # tokens per P1 tile
NG1 = NT // TOK     # 32 P1 tiles
SB = 16             # 128-token stat groups per P1 tile
RSB = 4             # P1 tiles per rs batch (64 groups -> one sqrt)

# 1-wait-per-instruction splitting for the TileContext exit drain: the
# walrus build in this container rejects instructions carrying >2 sem waits.
_MAXW = 1


class _TC(tile.TileContext):
    def _drain_and_barrier(self, tick_clock, wait_clock):
        nc = self.nc
        probe = nc.sync.nop(nofuse=True)
        wait_clock.add_sem_waits(
            probe.ins, ScopedClock({None: tick_clock.global_clock})
        )
        si = probe.ins.sync_info
        waits = list(si.on_wait) if si is not None else []
        if len(waits) > _MAXW:
            probe.ins.sync_info = mybir.SyncInfo(
                on_wait=waits[:_MAXW], on_update=list(si.on_update)
            )
            rest = waits[_MAXW:]
            for i in range(0, len(rest), _MAXW):
                w = nc.sync.nop(nofuse=True)
                w.ins.sync_info = mybir.SyncInfo(
                    on_wait=rest[i : i + _MAXW], on_update=[]
                )
        nc.sync.drain()
        nc.all_engine_barrier()
        popped = nc._tile_sem_poison_stack.pop()
        assert popped is self._sem_poison
        nc.clear_and_free_semaphores(list(self.sems.allocated().values()))
        nc.all_engine_barrier()


def _rs_from_stats(nc, pool, st, rs, ngrp, inv_n, eps_sb):
    """rs[:, :ngrp] = 1/sqrt(var + eps) from bn_stats partials st[128, ngrp, 6].

    bn_stats emits (count, mean, count*var) for even and odd elements; the
    exact combine for equal halves is
      var = (cv_e + cv_o) * inv_n + ((m_e - m_o)/2)^2.
    """
    d = pool.tile([128, ngrp], F32, tag="rsd")
    nc.vector.tensor_sub(out=d, in0=st[:, :, 1], in1=st[:, :, 4])
    nc.vector.tensor_scalar_mul(out=d, in0=d, scalar1=0.5)
    sq = pool.tile([128, ngrp], F32, tag="rssq")
    nc.vector.tensor_mul(out=sq, in0=d, in1=d)
    cv = pool.tile([128, ngrp], F32, tag="rscv")
    nc.vector.tensor_add(out=cv, in0=st[:, :, 2], in1=st[:, :, 5])
    var = pool.tile([128, ngrp], F32, tag="rsvar")
    nc.vector.scalar_tensor_tensor(
        out=var, in0=cv, scalar=inv_n, in1=sq, op0=ALU.mult, op1=ALU.add
    )
    nc.scalar.activation(out=rs, in_=var, func=AF.Sqrt, bias=eps_sb, scale=1.0)
    nc.vector.reciprocal(out=rs, in_=rs)


def _build(ctx, tc):
    nc = tc.nc

    # x_pre[p, (g, s, c)] = x token (g*1024 + s*128 + p) in column-half-major
    # order (host-shuffled), channel c.
    x_rows = nc.dram_tensor("x_rows", (128, NT * D // 128), F32, kind="ExternalInput").ap()
    w_pin = nc.dram_tensor("w_pin_t", (D, D), BF16, kind="ExternalInput").ap()
    w_gin = nc.dram_tensor("w_gin_t", (D, D), BF16, kind="ExternalInput").ap()
    w_pout = nc.dram_tensor("w_pout_t", (H, D), BF16, kind="ExternalInput").ap()
    w_gout = nc.dram_tensor("w_gout_t", (H, D), BF16, kind="ExternalInput").ap()
    out_cm = nc.dram_tensor("out_cm", (D, NT // 2), F32, kind="ExternalOutput").ap()

    persist = ctx.enter_context(tc.tile_pool(name="persist", bufs=1))
    # h_qh[p, r, c] = gated-h of token (r, qh*128 + p), channel c.
    h_q0 = persist.tile([128, N, D], BF16)
    h_q1 = persist.tile([128, N, D], BF16)
    w_pin_sb = persist.tile([D, D], BF16)
    w_gin_sb = persist.tile([D, D], BF16)
    w_pout_sb = persist.tile([H, D], BF16)
    w_gout_sb = persist.tile([H, D], BF16)
    eps_sb = persist.tile([128, 1], F32)
    nc.sync.dma_start(out=w_pin_sb, in_=w_pin)
    nc.sync.dma_start(out=w_gin_sb, in_=w_gin)
    nc.sync.dma_start(out=w_pout_sb, in_=w_pout)
    nc.sync.dma_start(out=w_gout_sb, in_=w_gout)
    nc.vector.memset(eps_sb, EPS)

    # ---------------- Phase 1: LN + gated down-projection ----------------
    with (
        tc.tile_pool(name="p1b", bufs=2 * RSB) as p1b,
        tc.tile_pool(name="p1st", bufs=2) as p1st,
        tc.tile_pool(name="p1rs", bufs=2) as p1rs,
        tc.tile_pool(name="p1t", bufs=RSB) as p1t,
        tc.tile_pool(name="p1h", bufs=2) as p1h,
        tc.tile_pool(name="p1hg", bufs=4) as p1hg,
        tc.tile_pool(name="p1p", bufs=2, space="PSUM") as p1p,
    ):
        sts = None
        xbs = []
        for g in range(NG1):
            gb = g % RSB
            if gb == 0:
                sts = p1st.tile([128, SB * RSB, 6], F32, tag="st")
                xbs = []
            # f32 -> bf16 cast rides the SWDGE load
            xb = p1b.tile([128, SB, D], BF16, tag="xb")
            nc.gpsimd.dma_start(
                out=xb,
                in_=x_rows[:, g * TOK : (g + 1) * TOK].rearrange(
                    "p (s c) -> p s c", s=SB
                ),
            )
            xbs.append(xb)
            for s in range(SB):
                nc.vector.bn_stats(out=sts[:, gb * SB + s, :], in_=xb[:, s, :])
            if gb == RSB - 1:
                rs = p1rs.tile([128, SB * RSB], F32, tag="rs")
                _rs_from_stats(nc, p1rs, sts, rs, SB * RSB, 1.0 / D, eps_sb)
                rsv = rs.rearrange("p (s one) -> p s one", one=1)
                # stage-major emission within the batch keeps each engine's
                # queue free of head-of-line stalls.
                for gi in range(RSB):  # scale in place (one broadcast TT)
                    rb = rsv[:, gi * SB : (gi + 1) * SB, :].broadcast_to(
                        (128, SB, D)
                    )
                    nc.vector.tensor_mul(out=xbs[gi], in0=xbs[gi], in1=rb)
                xTs = []
                for gi in range(RSB):
                    xT = p1t.tile([128, SB, D], BF16, tag="xT")
                    nc.sync.dma_start_transpose(
                        out=xT, in_=xbs[gi].rearrange("p s c -> p (s c)")
                    )
                    xTs.append(xT)
                for gi in range(RSB):
                    gg = g - (RSB - 1) + gi
                    xTf = xTs[gi].rearrange("p s c -> p (s c)")
                    hg = p1hg.tile([128, SB, D], BF16, tag="hg")
                    hgf = hg.rearrange("p s c -> p (s c)")
                    for u2 in range(2):
                        base = u2 * 1024
                        pp = p1p.tile([128, 1024], F32, tag="pp")
                        pg = p1p.tile([128, 1024], F32, tag="pg")
                        for hh in range(2):
                            so = slice(base + hh * 512, base + (hh + 1) * 512)
                            nc.tensor.matmul(
                                pp[:, hh * 512 : (hh + 1) * 512],
                                w_pin_sb, xTf[:, so], start=True, stop=True,
                            )
                        for hh in range(2):
                            so = slice(base + hh * 512, base + (hh + 1) * 512)
                            nc.tensor.matmul(
                                pg[:, hh * 512 : (hh + 1) * 512],
                                w_gin_sb, xTf[:, so], start=True, stop=True,
                            )
                        sg = p1h.tile([128, 1024], BF16, tag="sg")
                        nc.scalar.activation(out=sg, in_=pg, func=AF.Sigmoid)
                        ppb = p1h.tile([128, 1024], BF16, tag="ppb")
                        nc.scalar.activation(out=ppb, in_=pp, func=AF.Copy)
                        nc.vector.tensor_mul(
                            out=hgf[:, base : base + 1024], in0=ppb, in1=sg
                        )
                    # tokens gg*2048.. are rows r = gg*16.. of half gg//16
                    h_dst = h_q0 if gg < NG1 // 2 else h_q1
                    r0 = (gg % (NG1 // 2)) * SB
                    nc.sync.dma_start_transpose(
                        out=h_dst[:, r0 : r0 + SB, :], in_=hgf
                    )

    # ---------------- Phase 2: triangle matmuls ----------------
    hq = (h_q0, h_q1)
    with tc.tile_pool(name="p2tri", bufs=1) as p2tri:
        # tri[p, c, jh, i] = triangle-out channel c of token (i, jh*128 + p)
        tri = p2tri.tile([128, H, 2, 128], BF16)

        with (
            tc.tile_pool(name="p2s", bufs=3) as p2s,
            tc.tile_pool(name="p2t", bufs=3) as p2t,
            tc.tile_pool(name="p2e", bufs=3) as p2e,
            tc.tile_pool(name="p2p", bufs=4, space="PSUM") as p2p,
        ):
            # channels are processed 4 at a time so every xbar transpose
            # moves a [128, 1024] block (the per-op cost is ~fixed).
            def evac4(c_out, pss):
                ev = p2e.tile([128, 4, 256], BF16, tag="ev")
                for co, ps in enumerate(pss):
                    if co % 2 == 0:
                        nc.scalar.activation(out=ev[:, co, :], in_=ps,
                                             func=AF.Copy)
                    else:
                        nc.vector.tensor_copy(out=ev[:, co, :], in_=ps)
                nc.sync.dma_start_transpose(
                    out=tri[:, c_out : c_out + 4, :, :].rearrange(
                        "p c jh i -> p (c jh) i"
                    ),
                    in_=ev.rearrange("p a b -> p (a b)"),
                )

            for c4 in range(Q // 4):  # x1: a1=ch c, b1=ch Q+c
                os = []
                for co in range(4):
                    c = 4 * c4 + co
                    o1 = p2p.tile([128, 256], F32, tag="o1")
                    for kh in range(2):
                        nc.tensor.matmul(
                            o1,
                            hq[kh][:, 0:128, c],
                            hq[kh][:, 0:256, Q + c],
                            start=(kh == 0),
                            stop=(kh == 1),
                        )
                    os.append(o1)
                evac4(4 * c4, os)
            for c4 in range(Q // 4):  # x2: a2=ch 2Q+c, b2=ch 3Q+c
                # a2[k, i] lives at h_q0[p=i, r=k, 2Q+c]; b2[k, j] at
                # h_q(jh)[p=j%128, r=k, 3Q+c].  Stage contiguous copies
                # (spread over idle engines), then batched xbar transposes
                # put k on partitions.
                a2s = p2s.tile([128, 4, 256], BF16, tag="a2s")
                a2t = p2t.tile([128, 4, 2, 128], BF16, tag="a2t")  # [p,co,kb,i]
                for co in range(4):
                    src = h_q0[:, :, 2 * Q + 4 * c4 + co]
                    if co % 2 == 0:
                        nc.gpsimd.tensor_copy(out=a2s[:, co, :], in_=src)
                    else:
                        nc.vector.tensor_copy(out=a2s[:, co, :], in_=src)
                nc.sync.dma_start_transpose(
                    out=a2t.rearrange("p a b i -> p (a b) i"),
                    in_=a2s.rearrange("p a b -> p (a b)"),
                )
                b2t = p2t.tile([128, 4, 2, 2, 128], BF16, tag="b2t")  # [p,co,kb,jh,jl]
                for jh in range(2):
                    b2s = p2s.tile([128, 4, 256], BF16, tag="b2s")
                    for co in range(4):
                        src = hq[jh][:, :, 3 * Q + 4 * c4 + co]
                        if co % 2 == 0:
                            nc.gpsimd.tensor_copy(out=b2s[:, co, :], in_=src)
                        elif jh == 0:
                            nc.vector.tensor_copy(out=b2s[:, co, :], in_=src)
                        else:
                            nc.scalar.activation(
                                out=b2s[:, co, :], in_=src, func=AF.Copy
                            )
                    nc.sync.dma_start_transpose(
                        out=b2t[:, :, :, jh, :].rearrange("p a b i -> p (a b) i"),
                        in_=b2s.rearrange("p a b -> p (a b)"),
                    )
                for co in range(4):
                    o2 = p2p.tile([128, 256], F32, tag="o2")
                    for kb in range(2):
                        nc.tensor.matmul(
                            o2,
                            a2t[:, co, kb, :],
                            b2t[:, co, kb, :, :].rearrange("p a b -> p (a b)"),
                            start=(kb == 0),
                            stop=(kb == 1),
                        )
                    os2 = [o2] if co == 0 else os2 + [o2]
                evac4(Q + 4 * c4, os2)

        # ---------------- Phase 3: LN + gated up-projection ----------------
        ROWS = 8            # output rows per iteration (2048 tokens)
        NG3 = 128 // ROWS   # 16 iterations
        RSB3 = 4            # iterations per rs batch (64 groups)
        with (
            tc.tile_pool(name="p3st", bufs=2) as p3st,
            tc.tile_pool(name="p3rs", bufs=2) as p3rs,
            tc.tile_pool(name="p3n", bufs=3) as p3n,
            tc.tile_pool(name="p3r", bufs=2) as p3r,
            tc.tile_pool(name="p3o", bufs=2) as p3o,
            tc.tile_pool(name="p3ob", bufs=1) as p3ob,
            tc.tile_pool(name="p3p", bufs=2, space="PSUM") as p3p,
        ):
            tri_v = tri.rearrange("p c jh i -> p i jh c")
            GPI = 2 * ROWS  # stat groups per iteration
            sts3 = None
            for i8 in range(NG3):
                gb = i8 % RSB3
                if gb == 0:
                    sts3 = p3st.tile([128, GPI * RSB3, 6], F32, tag="st")
                for u in range(ROWS):
                    i = ROWS * i8 + u
                    for jh in range(2):
                        nc.vector.bn_stats(
                            out=sts3[:, gb * GPI + u * 2 + jh, :],
                            in_=tri_v[:, i, jh, :],
                        )
                if gb == RSB3 - 1:
                    rs3 = p3rs.tile([128, GPI * RSB3], F32, tag="rs")
                    _rs_from_stats(
                        nc, p3rs, sts3, rs3, GPI * RSB3, 1.0 / H, eps_sb
                    )
                    rsv = rs3.rearrange("p (s one) -> p s one", one=1)
                    hns = []
                    for ii in range(RSB3):
                        g3 = i8 - (RSB3 - 1) + ii
                        # hn[p, (u, jh), c] = normalized tri, channels 64..127
                        # garbage (never read: the matmuls contract K=64).
                        hn = p3n.tile([128, GPI, 128], BF16, tag="hn")
                        out4 = hn.rearrange("p (a b) c -> p a b c", b=2)[
                            :, :, :, 0:H
                        ]
                        in4 = tri_v[:, ROWS * g3 : ROWS * (g3 + 1), :, :]
                        rb = (
                            rsv[:, ii * GPI : (ii + 1) * GPI, :]
                            .broadcast_to((128, GPI, H))
                            .rearrange("p (a b) c -> p a b c", b=2)
                        )
                        nc.vector.tensor_mul(out=out4, in0=in4, in1=rb)
                        hns.append(hn)
                    rhss = []
                    for ii in range(RSB3):
                        rhs = p3r.tile([128, GPI, 128], BF16, tag="rhs")
                        nc.sync.dma_start_transpose(
                            out=rhs, in_=hns[ii].rearrange("p a c -> p (a c)")
                        )
                        rhss.append(rhs)
                    for ii in range(RSB3):
                        g3 = i8 - (RSB3 - 1) + ii
                        rhsf = rhss[ii].rearrange("p a c -> p (a c)")[0:H, :]
                        for u2 in range(2):
                            base = u2 * 1024
                            pp3 = p3p.tile([D, 1024], F32, tag="pp")
                            pg3 = p3p.tile([D, 1024], F32, tag="pg")
                            for hh in range(2):
                                so = slice(base + hh * 512, base + (hh + 1) * 512)
                                nc.tensor.matmul(
                                    pp3[:, hh * 512 : (hh + 1) * 512],
                                    w_pout_sb, rhsf[:, so],
                                    start=True, stop=True,
                                )
                            for hh in range(2):
                                so = slice(base + hh * 512, base + (hh + 1) * 512)
                                nc.tensor.matmul(
                                    pg3[:, hh * 512 : (hh + 1) * 512],
                                    w_gout_sb, rhsf[:, so],
                                    start=True, stop=True,
                                )
                            sg3 = p3o.tile([D, 1024], BF16, tag="sg")
                            nc.scalar.activation(out=sg3, in_=pg3, func=AF.Sigmoid)
                            ob = p3ob.tile([D, 1024], F32, tag="ob")
                            nc.vector.tensor_mul(out=ob, in0=pp3, in1=sg3)
                            nc.gpsimd.dma_start(
                                out=out_cm[
                                    :, g3 * 2048 + base : g3 * 2048 + base + 1024
                                ],
                                in_=ob,
                            )


_NC_CACHE = None


def _get_nc():
    global _NC_CACHE
    if _NC_CACHE is None:
        from contextlib import ExitStack

        nc = bass.Bass()
        with _TC(nc) as tc:
            with ExitStack() as ctx:
                _build(ctx, tc)
        _NC_CACHE = nc
    return _NC_CACHE


def kernel(
    x, mask, ln_in_w, ln_in_b, w_pin, w_gin, ln_out_w, ln_out_b, w_pout, w_gout,
    _spmd_kwargs=None,
):
    x = np.asarray(x, dtype=np.float32)
    w_pin = np.asarray(w_pin, dtype=np.float32)
    w_gin = np.asarray(w_gin, dtype=np.float32)
    w_pout = np.asarray(w_pout, dtype=np.float32)
    w_gout = np.asarray(w_gout, dtype=np.float32)

    # Fold LN mean-subtraction into the projection weights:
    #   LN(x) @ W.T == (x * rs) @ W'.T  with  W' = W - rowsum(W)/fan_in
    # (valid because both ln affines are identity and rs commutes).
    wp = w_pin - w_pin.sum(axis=1, keepdims=True) / D
    wg = w_gin - w_gin.sum(axis=1, keepdims=True) / D
    wpo = w_pout - w_pout.sum(axis=1, keepdims=True) / H
    wgo = w_gout - w_gout.sum(axis=1, keepdims=True) / H
    import ml_dtypes

    bf = lambda a: np.ascontiguousarray(a, dtype=ml_dtypes.bfloat16)
    w_common = {
        "w_pin_t": bf(wp.T),
        "w_gin_t": bf(wg.T),
        "w_pout_t": bf(wpo.T),
        "w_gout_t": bf(wgo.T),
    }

    in_maps = []
    for b in range(B):
        xb = np.ascontiguousarray(x[b])  # (N, N, D)
        xb_sw = np.ascontiguousarray(
            xb[np.r_[N // 2 : N, 0 : N // 2]][:, np.r_[N // 2 : N, 0 : N // 2]]
        )
        for xp in (xb, xb_sw):
            # column-half-major token order: n = (q//128)*32768 + r*128 + q%128
            x_ord = xp.reshape(N, 2, 128, D).transpose(1, 0, 2, 3)
            # device tile layout: x_pre[p, (g, s, c)], token = g*1024+s*128+p
            x_pre = np.ascontiguousarray(
                x_ord.reshape(NG1, SB, 128, D).transpose(2, 0, 1, 3)
            ).reshape(128, NT * D // 128)
            in_maps.append({"x_rows": x_pre, **w_common})

    nc = _get_nc()
    res = run_bass_kernel_spmd(
        nc, in_maps, core_ids=list(range(N_CORES)), **(_spmd_kwargs or {})
    )

    out = np.empty((B, N, N, D), dtype=np.float32)
    roll = np.r_[N // 2 : N, 0 : N // 2]
    for b in range(B):
        o0 = res.results[2 * b]["out_cm"].reshape(D, N // 2, N)
        o1 = res.results[2 * b + 1]["out_cm"].reshape(D, N // 2, N)
        out[b, : N // 2] = o0.transpose(1, 2, 0)
        # roll is an involution, so reorder columns directly
        out[b, N // 2 :] = o1.transpose(1, 2, 0)[:, roll, :]
    kernel._last_results = res
    return out
